# revision 22
# baseline (speedup 1.0000x reference)
"""AlignmentEncoder Trainium2 kernel.

Strategy: pure data parallel over batch (32 -> 4 examples x 8 cores).
Speaker biases are folded into the conv1 biases on device via tiny
matmuls (M_k = W1ksum@Wks etc. precomputed on host), so keys ship as
fp8 and queries as bf16 straight from host with no device-side
bias/cast passes.

Per core, per example:
  k-side:  conv(512->1024,k3) fp8 DoubleRow (x32 scale in W1/b1)
           -> relu (DVE, fp8 out) -> conv(1024->80,k1) fp8 DR (x8 W2,
           /256 folded into Identity drain) -> k (bf16)
           k2n = -temp * sum_c k^2 rides as row 80 of k_s.
  q-side:  conv(80->160,k3) bf16 (x32) -> relu fp8 -> conv(160->80)
           fp8 DR (x8) -> relu fp8 (x256) -> conv(80->80) fp8 (x256)
           -> q_s = 2*temp*q (bf16)
  attention: chunks of 256 T1 rows, partition p holds rows r0+2p,
           r0+2p+1; software-pipelined stages (exp of chunk c issues
           before ln of chunk c-2) and conv groups of example ex+1 are
           interleaved between chunks so no engine queue blocks.
           logits = q_s[:, r0+c::2].T @ k_s   (PSUM fp32)
           e1 = exp(logits) f32, s1 = row-sum (ACT)
           g  = e1 * prior -> bf16            (GPSIMD, whole chunk)
           lp = ln(g*r1 + 1e-8*r1) f32        (ACT)
           n  = g * maskmult bf16, s2 = row-sum (DVE)
           attn = n * (1/s2) bf16             (DVE)
prior in bf16; attn out bf16 (host casts up), lp out f32.
"""

import numpy as np
import ml_dtypes


def _ensure_paths():
    import sys
    try:
        import concourse  # noqa: F401
        return
    except ImportError:
        pass
    for p in ("/opt/trn_rl_repo", "/root/.axon_site/_ro/trn_rl_repo",
              "/root/.axon_site", "/opt/pypackages", "/root/.axon_site/_ro/pypackages"):
        if p not in sys.path:
            sys.path.append(p)
    import concourse  # noqa: F401


N_CORES = 8
B, BL = 32, 4
CM, CT, CA = 80, 512, 80
T1, T2 = 1600, 400
TEMP = 0.0005
BF16 = ml_dtypes.bfloat16
FP8 = ml_dtypes.float8_e4m3
S1K = 32.0   # W1k/b1k scale (k1_sb = 32*relu1)
S2K = 8.0    # W2k scale -> k2 psum = 256*conv2
S1Q = 32.0
S2Q = 8.0    # q2_sb = 256*relu2
S3Q = 256.0  # q3 psum = 65536*conv3

_CACHE = {}


def _build_nc():
    _ensure_paths()
    import concourse.bass as bass
    import concourse.bacc as bacc
    import concourse.mybir as mybir
    import concourse.tile as tile
    from contextlib import ExitStack

    f32 = mybir.dt.float32
    bf = mybir.dt.bfloat16
    f8 = mybir.dt.float8e4
    AF = mybir.ActivationFunctionType
    OP = mybir.AluOpType
    DR = mybir.MatmulPerfMode.DoubleRow

    nc = bacc.Bacc("TRN2", target_bir_lowering=False, debug=False,
                   enable_asserts=False)

    # ---- DRAM I/O ----
    d_q = nc.dram_tensor("queries", [BL, CM, T1 + 2], bf, kind="ExternalInput")
    d_k = nc.dram_tensor("keys", [BL, 128, 4, T2 + 2], f8, kind="ExternalInput")
    d_spk = nc.dram_tensor("spk", [128, 4, BL], bf, kind="ExternalInput")
    d_prior = nc.dram_tensor("prior", [BL, T1, T2], bf, kind="ExternalInput")
    d_pmask = nc.dram_tensor("pmask", [1, BL, T2], bf, kind="ExternalInput")

    d_mk = nc.dram_tensor("mk", [128, 4, 1024], f8, kind="ExternalInput")
    d_mq = nc.dram_tensor("mq", [128, 4, 160], bf, kind="ExternalInput")
    d_b0k = nc.dram_tensor("b0k", [128, 8], f32, kind="ExternalInput")
    d_b0q = nc.dram_tensor("b0q", [CM, 2], f32, kind="ExternalInput")
    d_wk1 = nc.dram_tensor("wk1", [128, 6, 2, 8, 128], f8, kind="ExternalInput")
    d_wk2 = nc.dram_tensor("wk2", [128, 4, 2, CA], f8, kind="ExternalInput")
    d_wq1 = nc.dram_tensor("wq1", [CM, 3, 160], bf, kind="ExternalInput")
    d_wq2 = nc.dram_tensor("wq2", [CM, 2, CA], f8, kind="ExternalInput")
    d_wq3 = nc.dram_tensor("wq3", [CM, CA], f8, kind="ExternalInput")
    d_bk2c = nc.dram_tensor("bk2c", [CA, 1], f32, kind="ExternalInput")
    d_bq2c = nc.dram_tensor("bq2c", [CA, 1], f32, kind="ExternalInput")
    d_bq3c = nc.dram_tensor("bq3c", [CA, 1], f32, kind="ExternalInput")

    d_attn = nc.dram_tensor("attn", [BL, T1, T2], bf, kind="ExternalOutput")
    d_lp = nc.dram_tensor("lp", [BL, T1, T2], f32, kind="ExternalOutput")

    with tile.TileContext(nc) as tc, ExitStack() as ctx:
        const = ctx.enter_context(tc.tile_pool(name="const", bufs=1))
        glob = ctx.enter_context(tc.tile_pool(name="glob", bufs=1))
        kk = ctx.enter_context(tc.tile_pool(name="kk", bufs=2))
        qq = ctx.enter_context(tc.tile_pool(name="qq", bufs=2))
        io = ctx.enter_context(tc.tile_pool(name="io", bufs=4))
        sm = ctx.enter_context(tc.tile_pool(name="sm", bufs=8))
        ps_mm = ctx.enter_context(
            tc.tile_pool(name="psmm", bufs=2, space=bass.MemorySpace.PSUM))
        ps_at = ctx.enter_context(
            tc.tile_pool(name="psat", bufs=4, space=bass.MemorySpace.PSUM))

        # ---- small/critical constants first (weight blobs come later) ----
        m_k = const.tile([128, 4, 1024], f8)
        nc.sync.dma_start(out=m_k[:], in_=d_mk.ap())
        m_q = const.tile([128, 4, 160], bf)
        nc.sync.dma_start(out=m_q[:], in_=d_mq.ap())
        b0k = const.tile([128, 8], f32)
        nc.sync.dma_start(out=b0k[:], in_=d_b0k.ap())
        b0q = const.tile([CM, 2], f32)
        nc.sync.dma_start(out=b0q[:], in_=d_b0q.ap())
        b_k2c = const.tile([CA, 1], f32)
        nc.sync.dma_start(out=b_k2c[:], in_=d_bk2c.ap())
        b_q2c = const.tile([CA, 1], f32)
        nc.sync.dma_start(out=b_q2c[:], in_=d_bq2c.ap())
        b_q3c = const.tile([CA, 1], f32)
        nc.sync.dma_start(out=b_q3c[:], in_=d_bq3c.ap())
        spk_sb = glob.tile([128, 4, BL], bf)
        nc.sync.dma_start(out=spk_sb[:], in_=d_spk.ap())
        mrow = glob.tile([1, BL, T2], bf)
        nc.sync.dma_start(out=mrow[:], in_=d_pmask.ap())

        keys8 = glob.tile([128, BL, 4, T2 + 2], f8)
        for ex in range(BL):
            nc.sync.dma_start(out=keys8[:, ex, :, :], in_=d_k.ap()[ex])
        q_sb = glob.tile([CM, BL, T1 + 2], bf)
        for ex in range(BL):
            nc.sync.dma_start(out=q_sb[:, ex, :], in_=d_q.ap()[ex])

        w_q1 = const.tile([CM, 3, 160], bf)
        nc.sync.dma_start(out=w_q1[:], in_=d_wq1.ap())
        w_q2 = const.tile([CM, 2, CA], f8)
        nc.sync.dma_start(out=w_q2[:], in_=d_wq2.ap())
        w_q3 = const.tile([CM, CA], f8)
        nc.sync.dma_start(out=w_q3[:], in_=d_wq3.ap())
        w_k2 = const.tile([128, 4, 2, CA], f8)
        nc.sync.dma_start(out=w_k2[:], in_=d_wk2.ap())
        w_k1 = const.tile([128, 6, 2, 8, 128], f8)
        nc.sync.dma_start(out=w_k1[:], in_=d_wk1.ap())

        ld = mybir.InstLoadActFuncSet(name=nc.get_next_instruction_name(),
                                      act_func_set_id=6, ins=[], outs=[])
        nc.scalar.add_instruction(ld)

        ones_col = const.tile([CM, 1], bf)
        nc.vector.memset(ones_col[:], 1.0)
        ones_row = const.tile([1, T1], bf)
        nc.vector.memset(ones_row[:], 1.0)

        maskm = glob.tile([128, BL, T2], bf)
        for ex in range(BL):
            nc.gpsimd.partition_broadcast(maskm[:, ex, :], mrow[0:1, ex, :])

        # ---- per-example conv1 biases via folded speaker projection ----
        # bk1e[:, mt, ex] = 32*(b0k + M_k @ spk)
        bk1e = glob.tile([128, 8, BL], f32)
        for mt in range(8):
            ps = ps_mm.tile([128, 2, 512], f32, tag="mm")
            for kt in range(4):
                nc.tensor.matmul(ps[:, 0, 0:BL], m_k[:, kt, mt * 128:(mt + 1) * 128],
                                 spk_sb[:, kt, :], start=(kt == 0), stop=(kt == 3))
            nc.vector.tensor_scalar(out=bk1e[:, mt, :], in0=ps[:, 0, 0:BL],
                                    scalar1=S1K, scalar2=b0k[:, mt:mt + 1],
                                    op0=OP.mult, op1=OP.add)
        bq1e = glob.tile([CM, 2, BL], f32)
        for grp in range(2):
            ps = ps_mm.tile([128, 2, 512], f32, tag="mm")
            for kt in range(4):
                nc.tensor.matmul(ps[0:CM, 0, 0:BL], m_q[:, kt, grp * 80:grp * 80 + 80],
                                 spk_sb[:, kt, :], start=(kt == 0), stop=(kt == 3))
            nc.vector.tensor_scalar(out=bq1e[:, grp, :], in0=ps[0:CM, 0, 0:BL],
                                    scalar1=S1Q, scalar2=b0q[:, grp:grp + 1],
                                    op0=OP.mult, op1=OP.add)

        qs_tiles = []
        for i in range(2):
            qs = glob.tile([81, T1], bf, tag=f"qs{i}")
            nc.sync.dma_start(out=qs[80:81, :], in_=ones_row[0:1, 0:T1])
            qs_tiles.append(qs)

        # ---------- conv work for one example, as a list of closures ----------
        def conv_groups(ex):
            groups = []
            k1_sb = kk.tile([128, 8, T2], f8, tag="k1")
            k_s = kk.tile([81, T2], bf, tag="ks")
            ksq = kk.tile([CM, T2], bf, tag="ksq")
            k2row = kk.tile([1, T2], bf, tag="k2row")
            q1_sb = qq.tile([CM, 2, T1], f8, tag="q1")
            q2_sb = qq.tile([CA, T1], f8, tag="q2")
            q_s = qs_tiles[ex % 2]

            def k1_group(mp):
                def run():
                    ps = ps_mm.tile([128, 2, 512], f32, tag="mm")
                    for half in range(2):
                        mt = 2 * mp + half
                        n_mm = 0
                        for cp in range(2):
                            for dt in range(3):
                                n_mm += 1
                                nc.tensor.matmul(
                                    ps[:, half, 0:T2], w_k1[:, cp * 3 + dt, :, mt, :],
                                    keys8[:, ex, 2 * cp:2 * cp + 2, dt:dt + T2],
                                    start=(n_mm == 1), stop=(n_mm == 6),
                                    perf_mode=DR)
                        nc.vector.tensor_scalar(
                            out=k1_sb[:, mt, :], in0=ps[:, half, 0:T2],
                            scalar1=bk1e[:, mt, ex:ex + 1], scalar2=0.0,
                            op0=OP.add, op1=OP.max)
                return run

            for mp in range(4):
                groups.append(k1_group(mp))

            def k2_group():
                ps = ps_mm.tile([128, 2, 512], f32, tag="mm")
                for kp in range(4):
                    nc.tensor.matmul(ps[0:CA, 0, 0:T2], w_k2[:, kp, :, :],
                                     k1_sb[:, 2 * kp:2 * kp + 2, :],
                                     start=(kp == 0), stop=(kp == 3), perf_mode=DR)
                nc.scalar.activation(out=k_s[0:CA, :], in_=ps[0:CA, 0, 0:T2],
                                     func=AF.Identity, scale=1.0 / (S1K * S2K),
                                     bias=b_k2c[:, 0:1])
            groups.append(k2_group)

            def k2n_group():
                nc.gpsimd.tensor_tensor(out=ksq[:], in0=k_s[0:CA, :],
                                        in1=k_s[0:CA, :], op=OP.mult)
                ps2 = ps_mm.tile([128, 2, 512], f32, tag="mm")
                nc.tensor.matmul(ps2[0:1, 0, 0:T2], ones_col[:, 0:1], ksq[:],
                                 start=True, stop=True)
                nc.vector.tensor_scalar(out=k2row[:], in0=ps2[0:1, 0, 0:T2],
                                        scalar1=-TEMP, scalar2=None, op0=OP.mult)
                nc.sync.dma_start(out=k_s[80:81, :], in_=k2row[:])
            groups.append(k2n_group)

            def q1_group(grp, sp):
                def run():
                    ps = ps_mm.tile([128, 2, 512], f32, tag="mm")
                    for sub in range(2):
                        base = (sp * 2 + sub) * 400
                        for dt in range(3):
                            nc.tensor.matmul(
                                ps[0:CM, sub, 0:400],
                                w_q1[:, dt, grp * 80:grp * 80 + 80],
                                q_sb[:, ex, dt + base:dt + base + 400],
                                start=(dt == 0), stop=(dt == 2))
                    nc.vector.tensor_scalar(
                        out=q1_sb[:, grp, sp * 800:sp * 800 + 800]
                        .rearrange("p (s t) -> p s t", s=2),
                        in0=ps[0:CM, :, 0:400],
                        scalar1=bq1e[0:CM, grp, ex:ex + 1], scalar2=0.0,
                        op0=OP.add, op1=OP.max)
                return run

            for grp in range(2):
                groups.append(q1_group(grp, 0))

            def q2_group(sp):
                def run():
                    ps = ps_mm.tile([128, 2, 512], f32, tag="mm")
                    for sub in range(2):
                        base = (sp * 2 + sub) * 400
                        nc.tensor.matmul(ps[0:CA, sub, 0:400], w_q2[:, :, :],
                                         q1_sb[:, 0:2, base:base + 400],
                                         start=True, stop=True, perf_mode=DR)
                    nc.scalar.activation(
                        out=q2_sb[:, sp * 800:sp * 800 + 800]
                        .rearrange("p (s t) -> p s t", s=2),
                        in_=ps[0:CA, :, 0:400], func=AF.Relu,
                        bias=b_q2c[:, 0:1])
                return run

            groups.append(q2_group(0))

            def q3_group(sp):
                def run():
                    ps = ps_mm.tile([128, 2, 512], f32, tag="mm")
                    for sub in range(2):
                        base = (sp * 2 + sub) * 400
                        nc.tensor.matmul(ps[0:CA, sub, 0:400], w_q3[:],
                                         q2_sb[:, base:base + 400],
                                         start=True, stop=True)
                    nc.scalar.activation(
                        out=q_s[0:CA, sp * 800:sp * 800 + 800]
                        .rearrange("p (s t) -> p s t", s=2),
                        in_=ps[0:CA, :, 0:400], func=AF.Identity,
                        scale=2.0 * TEMP / (S1Q * S2Q * S3Q),
                        bias=b_q3c[:, 0:1])
                return run

            groups.append(q3_group(0))
            tail = [q1_group(0, 1), q1_group(1, 1), q2_group(1), q3_group(1)]
            return groups, tail, (k_s, q_s)

        # ---------- attention for one example, pipelined + interleaved ----------
        CHUNKS = [(r0, 2, 128) for r0 in range(0, 1536, 256)] + [(1536, 1, 64)]

        def attention(ex, k_s, q_s, pending):
            st = {}

            def stage0(i):
                r0, cn, prow = CHUNKS[i]
                nrows = cn * prow
                t = {}
                t["pr"] = io.tile([128, 2, T2], bf, tag="pr", name="pr")
                t["e1"] = io.tile([128, 2, T2], f32, tag="e1", name="e1")
                t["g"] = io.tile([128, 2, T2], bf, tag="g", name="g")
                t["n"] = io.tile([128, 2, T2], bf, tag="n", name="n")
                t["lp"] = io.tile([128, 2, T2], f32, tag="lp", name="lp")
                t["at"] = io.tile([128, 2, T2], bf, tag="at", name="at")
                t["s1"] = sm.tile([128, 2], f32, tag="s1", name="s1")
                t["s2"] = sm.tile([128, 2], f32, tag="s2", name="s2")
                t["r1"] = sm.tile([128, 2], f32, tag="r1", name="r1")
                t["r1e"] = sm.tile([128, 2], f32, tag="r1e", name="r1e")
                t["r2"] = sm.tile([128, 2], f32, tag="r2", name="r2")
                st[i] = t
                nc.sync.dma_start(
                    out=t["pr"][0:prow, 0:cn, :],
                    in_=d_prior.ap()[ex, r0:r0 + nrows, :]
                    .rearrange("(p c) t -> p c t", c=cn))
                for c in range(cn):
                    ps = ps_at.tile([128, 512], f32, tag="att")
                    nc.tensor.matmul(ps[0:prow, 0:T2],
                                     q_s[:, r0 + c:r0 + nrows:cn],
                                     k_s[:], start=True, stop=True)
                    nc.scalar.activation(out=t["e1"][0:prow, c, :],
                                         in_=ps[0:prow, 0:T2], func=AF.Exp,
                                         accum_out=t["s1"][0:prow, c:c + 1])

            def stage1(i):
                r0, cn, prow = CHUNKS[i]
                t = st[i]
                # split g across gpsimd/DVE: shortens the per-chunk serial
                # chain (gpsimd is the slowest per-pass engine)
                nc.gpsimd.tensor_tensor(out=t["g"][0:prow, 0:1, :],
                                        in0=t["e1"][0:prow, 0:1, :],
                                        in1=t["pr"][0:prow, 0:1, :], op=OP.mult)
                nc.vector.reciprocal(out=t["r1"][0:prow, 0:cn],
                                     in_=t["s1"][0:prow, 0:cn])
                nc.vector.tensor_scalar(out=t["r1e"][0:prow, 0:cn],
                                        in0=t["r1"][0:prow, 0:cn],
                                        scalar1=1e-8, scalar2=None, op0=OP.mult)
                if cn == 2:
                    nc.vector.tensor_tensor(out=t["g"][0:prow, 1:2, :],
                                            in0=t["e1"][0:prow, 1:2, :],
                                            in1=t["pr"][0:prow, 1:2, :],
                                            op=OP.mult)

            def stage2(i):
                r0, cn, prow = CHUNKS[i]
                nrows = cn * prow
                t = st.pop(i)
                for c in range(cn):
                    nc.scalar.activation(out=t["lp"][0:prow, c, :],
                                         in_=t["g"][0:prow, c, :], func=AF.Ln,
                                         scale=t["r1"][0:prow, c:c + 1],
                                         bias=t["r1e"][0:prow, c:c + 1])
                    nc.vector.scalar_tensor_tensor(
                        out=t["n"][0:prow, c, :], in0=t["g"][0:prow, c, :],
                        scalar=1.0, in1=maskm[0:prow, ex, :],
                        op0=OP.mult, op1=OP.mult,
                        accum_out=t["s2"][0:prow, c:c + 1])
                nc.sync.dma_start(
                    out=d_lp.ap()[ex, r0:r0 + nrows, :]
                    .rearrange("(p c) t -> p c t", c=cn),
                    in_=t["lp"][0:prow, 0:cn, :])
                nc.vector.reciprocal(out=t["r2"][0:prow, 0:cn],
                                     in_=t["s2"][0:prow, 0:cn])
                for c in range(cn):
                    nc.vector.tensor_scalar(out=t["at"][0:prow, c, :],
                                            in0=t["n"][0:prow, c, :],
                                            scalar1=t["r2"][0:prow, c:c + 1],
                                            scalar2=None, op0=OP.mult)
                nc.sync.dma_start(
                    out=d_attn.ap()[ex, r0:r0 + nrows, :]
                    .rearrange("(p c) t -> p c t", c=cn),
                    in_=t["at"][0:prow, 0:cn, :])

            nch = len(CHUNKS)
            for i in range(nch + 2):
                if i < nch:
                    stage0(i)
                if 1 <= i < nch + 1:
                    stage1(i - 1)
                if i >= 2:
                    stage2(i - 2)
                # interleave ~2 conv groups of the next example per chunk
                for _ in range(2):
                    if pending:
                        pending.pop(0)()

        # ---------- main schedule ----------
        head0, tail0, tiles0 = conv_groups(0)
        for g in head0:
            g()
        cur = tiles0
        pending = list(tail0)
        for ex in range(BL):
            if ex + 1 < BL:
                nxt_head, nxt_tail, nxt_tiles = conv_groups(ex + 1)
                pending += nxt_head + nxt_tail
            else:
                nxt_tiles = None
            attention(ex, cur[0], cur[1], pending)
            for g in pending:
                g()
            pending = []
            cur = nxt_tiles

    nc.compile()
    return nc


def get_nc():
    if "nc" not in _CACHE:
        _CACHE["nc"] = _build_nc()
    return _CACHE["nc"]


def prep_in_maps(inputs):
    q = np.asarray(inputs["queries"], np.float32)
    k = np.asarray(inputs["keys"], np.float32)
    mask = np.asarray(inputs["mask"])
    prior = np.asarray(inputs["attn_prior"], np.float32)
    spk = np.asarray(inputs["speaker_embed"], np.float32)

    def f32(x):
        return np.ascontiguousarray(np.asarray(x, np.float32))

    def bf(x):
        return np.ascontiguousarray(np.asarray(x, np.float32).astype(BF16))

    def fp8(x):
        return np.ascontiguousarray(np.asarray(x, np.float32).astype(FP8))

    Wk1, bk1 = f32(inputs["Wk1"]), f32(inputs["bk1"])
    Wk2, bk2 = f32(inputs["Wk2"]), f32(inputs["bk2"])
    Wq1, bq1 = f32(inputs["Wq1"]), f32(inputs["bq1"])
    Wq2, bq2 = f32(inputs["Wq2"]), f32(inputs["bq2"])
    Wq3, bq3 = f32(inputs["Wq3"]), f32(inputs["bq3"])
    Wks, bks = f32(inputs["Wks"]), f32(inputs["bks"])
    Wqs, bqs = f32(inputs["Wqs"]), f32(inputs["bqs"])

    # speaker-bias folding: conv1(x + s) = conv1(x) + W1sum @ s
    W1ksum = Wk1.sum(axis=2)              # [1024, 512]
    Mk = W1ksum @ Wks                      # [1024, 512] (spk -> bias)
    b0k = bk1 + W1ksum @ bks               # [1024]
    W1qsum = Wq1.sum(axis=2)               # [160, 80]
    Mq = W1qsum @ Wqs                      # [160, 512]
    b0q = bq1 + W1qsum @ bqs               # [160]

    wk1 = fp8((S1K * Wk1).reshape(8, 128, 2, 2, 128, 3)
              .transpose(4, 2, 5, 3, 0, 1).reshape(128, 6, 2, 8, 128))
    wk2 = fp8((S2K * Wk2[:, :, 0]).reshape(CA, 4, 2, 128).transpose(3, 1, 2, 0))
    wq1 = bf((S1Q * Wq1).transpose(1, 2, 0))
    wq2 = fp8((S2Q * Wq2[:, :, 0]).T.reshape(2, CM, CA).transpose(1, 0, 2))
    wq3 = fp8((S3Q * Wq3[:, :, 0]).T)
    mk_h = fp8(Mk.T.reshape(4, 128, 1024).transpose(1, 0, 2))
    mq_h = bf(Mq.T.reshape(4, 128, 160).transpose(1, 0, 2))
    b0k_h = f32(S1K * b0k.reshape(8, 128).T)
    b0q_h = f32(S1Q * b0q.reshape(2, CM).T)
    bk2c = f32(bk2[:, None])
    bq2c = f32(S1Q * S2Q * bq2[:, None])
    bq3c = f32(2.0 * TEMP * bq3[:, None])

    q_t = np.zeros((B, CM, T1 + 2), np.float32)
    q_t[:, :, 1:T1 + 1] = q
    k_t = np.zeros((B, CT, T2 + 2), np.float32)
    k_t[:, :, 1:T2 + 1] = k
    q_bf = q_t.astype(BF16)
    k_f8 = k_t.astype(FP8)
    spk_bf = spk.astype(BF16)
    prior_bf = prior.astype(BF16)
    pmask = (~mask[:, :, 0]).astype(np.float32).astype(BF16)

    weights = dict(wk1=wk1, wk2=wk2, wq1=wq1, wq2=wq2, wq3=wq3,
                   mk=mk_h, mq=mq_h, b0k=b0k_h, b0q=b0q_h,
                   bk2c=bk2c, bq2c=bq2c, bq3c=bq3c)
    in_maps = []
    for c in range(N_CORES):
        sl = slice(c * BL, (c + 1) * BL)
        m = {"queries": np.ascontiguousarray(q_bf[sl]),
             "keys": np.ascontiguousarray(
                 k_f8[sl].reshape(BL, 4, 128, T2 + 2).transpose(0, 2, 1, 3)),
             "spk": np.ascontiguousarray(
                 spk_bf[sl].reshape(BL, 4, 128).transpose(2, 1, 0)),
             "prior": np.ascontiguousarray(prior_bf[sl]),
             "pmask": np.ascontiguousarray(pmask[sl][None])}
        m.update(weights)
        in_maps.append(m)
    return in_maps


def run_on_hw(inputs, trace=False, trace_kwargs=None):
    _ensure_paths()
    from concourse.bass_utils import run_bass_kernel_spmd
    nc = get_nc()
    in_maps = prep_in_maps(inputs)
    res = run_bass_kernel_spmd(nc, in_maps, core_ids=list(range(N_CORES)),
                               trace=trace, **(trace_kwargs or {}))
    attn = np.empty((B, 1, T1, T2), np.float32)
    lp = np.empty((B, 1, T1, T2), np.float32)
    for c in range(N_CORES):
        attn[c * BL:(c + 1) * BL, 0] = res.results[c]["attn"].astype(np.float32)
        lp[c * BL:(c + 1) * BL, 0] = res.results[c]["lp"].astype(np.float32)
    return (attn, lp), res


def kernel(**inputs):
    (attn, lp), _ = run_on_hw(inputs, trace=False)
    return attn, lp


# revision 23
# speedup vs baseline: 1.0006x; 1.0006x over previous
"""AlignmentEncoder Trainium2 kernel.

Strategy: pure data parallel over batch (32 -> 4 examples x 8 cores).
Speaker biases are folded into the conv1 biases on device via tiny
matmuls (M_k = W1ksum@Wks etc. precomputed on host), so keys ship as
fp8 and queries as bf16 straight from host with no device-side
bias/cast passes.

Per core, per example:
  k-side:  conv(512->1024,k3) fp8 DoubleRow (x32 scale in W1/b1)
           -> relu (DVE, fp8 out) -> conv(1024->80,k1) fp8 DR (x8 W2,
           /256 folded into Identity drain) -> k (bf16)
           k2n = -temp * sum_c k^2 rides as row 80 of k_s.
  q-side:  conv(80->160,k3) bf16 (x32) -> relu fp8 -> conv(160->80)
           fp8 DR (x8) -> relu fp8 (x256) -> conv(80->80) fp8 (x256)
           -> q_s = 2*temp*q (bf16)
  attention: chunks of 256 T1 rows, partition p holds rows r0+2p,
           r0+2p+1; software-pipelined stages (exp of chunk c issues
           before ln of chunk c-2) and conv groups of example ex+1 are
           interleaved between chunks so no engine queue blocks.
           logits = q_s[:, r0+c::2].T @ k_s   (PSUM fp32)
           e1 = exp(logits) f32, s1 = row-sum (ACT)
           g  = e1 * prior -> bf16            (GPSIMD, whole chunk)
           lp = ln(g*r1 + 1e-8*r1) f32        (ACT)
           n  = g * maskmult bf16, s2 = row-sum (DVE)
           attn = n * (1/s2) bf16             (DVE)
prior in bf16; attn out bf16 (host casts up), lp out f32.
"""

import numpy as np
import ml_dtypes


def _ensure_paths():
    import sys
    try:
        import concourse  # noqa: F401
        return
    except ImportError:
        pass
    for p in ("/opt/trn_rl_repo", "/root/.axon_site/_ro/trn_rl_repo",
              "/root/.axon_site", "/opt/pypackages", "/root/.axon_site/_ro/pypackages"):
        if p not in sys.path:
            sys.path.append(p)
    import concourse  # noqa: F401


N_CORES = 8
B, BL = 32, 4
CM, CT, CA = 80, 512, 80
T1, T2 = 1600, 400
TEMP = 0.0005
BF16 = ml_dtypes.bfloat16
FP8 = ml_dtypes.float8_e4m3
S1K = 32.0   # W1k/b1k scale (k1_sb = 32*relu1)
S2K = 8.0    # W2k scale -> k2 psum = 256*conv2
S1Q = 32.0
S2Q = 8.0    # q2_sb = 256*relu2
S3Q = 256.0  # q3 psum = 65536*conv3

_CACHE = {}


def _build_nc():
    _ensure_paths()
    import concourse.bass as bass
    import concourse.bacc as bacc
    import concourse.mybir as mybir
    import concourse.tile as tile
    from contextlib import ExitStack

    f32 = mybir.dt.float32
    bf = mybir.dt.bfloat16
    f8 = mybir.dt.float8e4
    AF = mybir.ActivationFunctionType
    OP = mybir.AluOpType
    DR = mybir.MatmulPerfMode.DoubleRow

    nc = bacc.Bacc("TRN2", target_bir_lowering=False, debug=False,
                   enable_asserts=False)

    # ---- DRAM I/O ----
    d_q = nc.dram_tensor("queries", [BL, CM, T1 + 2], bf, kind="ExternalInput")
    d_k = nc.dram_tensor("keys", [BL, 128, 4, T2 + 2], f8, kind="ExternalInput")
    d_spk = nc.dram_tensor("spk", [128, 4, BL], bf, kind="ExternalInput")
    d_prior = nc.dram_tensor("prior", [BL, T1, T2], bf, kind="ExternalInput")
    d_pmask = nc.dram_tensor("pmask", [1, BL, T2], bf, kind="ExternalInput")

    d_mk = nc.dram_tensor("mk", [128, 4, 1024], f8, kind="ExternalInput")
    d_mq = nc.dram_tensor("mq", [128, 4, 160], bf, kind="ExternalInput")
    d_b0k = nc.dram_tensor("b0k", [128, 8], f32, kind="ExternalInput")
    d_b0q = nc.dram_tensor("b0q", [CM, 2], f32, kind="ExternalInput")
    d_wk1 = nc.dram_tensor("wk1", [128, 6, 2, 8, 128], f8, kind="ExternalInput")
    d_wk2 = nc.dram_tensor("wk2", [128, 4, 2, CA], f8, kind="ExternalInput")
    d_wq1 = nc.dram_tensor("wq1", [CM, 3, 160], bf, kind="ExternalInput")
    d_wq2 = nc.dram_tensor("wq2", [CM, 2, CA], f8, kind="ExternalInput")
    d_wq3 = nc.dram_tensor("wq3", [CM, CA], f8, kind="ExternalInput")
    d_bk2c = nc.dram_tensor("bk2c", [CA, 1], f32, kind="ExternalInput")
    d_bq2c = nc.dram_tensor("bq2c", [CA, 1], f32, kind="ExternalInput")
    d_bq3c = nc.dram_tensor("bq3c", [CA, 1], f32, kind="ExternalInput")

    d_attn = nc.dram_tensor("attn", [BL, T1, T2], bf, kind="ExternalOutput")
    d_lp = nc.dram_tensor("lp", [BL, T1, T2], f32, kind="ExternalOutput")

    with tile.TileContext(nc) as tc, ExitStack() as ctx:
        const = ctx.enter_context(tc.tile_pool(name="const", bufs=1))
        glob = ctx.enter_context(tc.tile_pool(name="glob", bufs=1))
        kk = ctx.enter_context(tc.tile_pool(name="kk", bufs=2))
        qq = ctx.enter_context(tc.tile_pool(name="qq", bufs=2))
        io = ctx.enter_context(tc.tile_pool(name="io", bufs=4))
        sm = ctx.enter_context(tc.tile_pool(name="sm", bufs=8))
        ps_mm = ctx.enter_context(
            tc.tile_pool(name="psmm", bufs=2, space=bass.MemorySpace.PSUM))
        ps_at = ctx.enter_context(
            tc.tile_pool(name="psat", bufs=4, space=bass.MemorySpace.PSUM))

        # ---- small/critical constants first (weight blobs come later) ----
        m_k = const.tile([128, 4, 1024], f8)
        nc.sync.dma_start(out=m_k[:], in_=d_mk.ap())
        m_q = const.tile([128, 4, 160], bf)
        nc.sync.dma_start(out=m_q[:], in_=d_mq.ap())
        b0k = const.tile([128, 8], f32)
        nc.sync.dma_start(out=b0k[:], in_=d_b0k.ap())
        b0q = const.tile([CM, 2], f32)
        nc.sync.dma_start(out=b0q[:], in_=d_b0q.ap())
        b_k2c = const.tile([CA, 1], f32)
        nc.sync.dma_start(out=b_k2c[:], in_=d_bk2c.ap())
        b_q2c = const.tile([CA, 1], f32)
        nc.sync.dma_start(out=b_q2c[:], in_=d_bq2c.ap())
        b_q3c = const.tile([CA, 1], f32)
        nc.sync.dma_start(out=b_q3c[:], in_=d_bq3c.ap())
        spk_sb = glob.tile([128, 4, BL], bf)
        nc.sync.dma_start(out=spk_sb[:], in_=d_spk.ap())
        mrow = glob.tile([1, BL, T2], bf)
        nc.sync.dma_start(out=mrow[:], in_=d_pmask.ap())

        keys8 = glob.tile([128, BL, 4, T2 + 2], f8)
        for ex in range(BL):
            nc.sync.dma_start(out=keys8[:, ex, :, :], in_=d_k.ap()[ex])
        q_sb = glob.tile([CM, BL, T1 + 2], bf)
        for ex in range(BL):
            nc.sync.dma_start(out=q_sb[:, ex, :], in_=d_q.ap()[ex])

        w_q1 = const.tile([CM, 3, 160], bf)
        nc.sync.dma_start(out=w_q1[:], in_=d_wq1.ap())
        w_q2 = const.tile([CM, 2, CA], f8)
        nc.sync.dma_start(out=w_q2[:], in_=d_wq2.ap())
        w_q3 = const.tile([CM, CA], f8)
        nc.sync.dma_start(out=w_q3[:], in_=d_wq3.ap())
        w_k2 = const.tile([128, 4, 2, CA], f8)
        nc.sync.dma_start(out=w_k2[:], in_=d_wk2.ap())
        w_k1 = const.tile([128, 6, 2, 8, 128], f8)
        nc.sync.dma_start(out=w_k1[:], in_=d_wk1.ap())

        ld = mybir.InstLoadActFuncSet(name=nc.get_next_instruction_name(),
                                      act_func_set_id=6, ins=[], outs=[])
        nc.scalar.add_instruction(ld)

        ones_col = const.tile([CM, 1], bf)
        nc.vector.memset(ones_col[:], 1.0)
        ones_row = const.tile([1, T1], bf)
        nc.vector.memset(ones_row[:], 1.0)

        maskm = glob.tile([128, BL, T2], bf)
        for ex in range(BL):
            nc.gpsimd.partition_broadcast(maskm[:, ex, :], mrow[0:1, ex, :])

        # ---- per-example conv1 biases via folded speaker projection ----
        # bk1e[:, mt, ex] = 32*(b0k + M_k @ spk)
        bk1e = glob.tile([128, 8, BL], f32)
        for mt in range(8):
            ps = ps_mm.tile([128, 2, 512], f32, tag="mm")
            for kt in range(4):
                nc.tensor.matmul(ps[:, 0, 0:BL], m_k[:, kt, mt * 128:(mt + 1) * 128],
                                 spk_sb[:, kt, :], start=(kt == 0), stop=(kt == 3))
            nc.vector.tensor_scalar(out=bk1e[:, mt, :], in0=ps[:, 0, 0:BL],
                                    scalar1=S1K, scalar2=b0k[:, mt:mt + 1],
                                    op0=OP.mult, op1=OP.add)
        bq1e = glob.tile([CM, 2, BL], f32)
        for grp in range(2):
            ps = ps_mm.tile([128, 2, 512], f32, tag="mm")
            for kt in range(4):
                nc.tensor.matmul(ps[0:CM, 0, 0:BL], m_q[:, kt, grp * 80:grp * 80 + 80],
                                 spk_sb[:, kt, :], start=(kt == 0), stop=(kt == 3))
            nc.vector.tensor_scalar(out=bq1e[:, grp, :], in0=ps[0:CM, 0, 0:BL],
                                    scalar1=S1Q, scalar2=b0q[:, grp:grp + 1],
                                    op0=OP.mult, op1=OP.add)

        qs_tiles = []
        for i in range(2):
            qs = glob.tile([81, T1], bf, tag=f"qs{i}")
            nc.sync.dma_start(out=qs[80:81, :], in_=ones_row[0:1, 0:T1])
            qs_tiles.append(qs)

        # ---------- conv work for one example, as a list of closures ----------
        def conv_groups(ex):
            groups = []
            k1_sb = kk.tile([128, 8, T2], f8, tag="k1")
            k_s = kk.tile([81, T2], bf, tag="ks")
            ksq = kk.tile([CM, T2], bf, tag="ksq")
            k2row = kk.tile([1, T2], bf, tag="k2row")
            q1_sb = qq.tile([CM, 2, T1], f8, tag="q1")
            q2_sb = qq.tile([CA, T1], f8, tag="q2")
            q_s = qs_tiles[ex % 2]

            def k1_group(mp):
                def run():
                    ps = ps_mm.tile([128, 2, 512], f32, tag="mm")
                    for half in range(2):
                        mt = 2 * mp + half
                        n_mm = 0
                        for cp in range(2):
                            for dt in range(3):
                                n_mm += 1
                                nc.tensor.matmul(
                                    ps[:, half, 0:T2], w_k1[:, cp * 3 + dt, :, mt, :],
                                    keys8[:, ex, 2 * cp:2 * cp + 2, dt:dt + T2],
                                    start=(n_mm == 1), stop=(n_mm == 6),
                                    perf_mode=DR)
                        nc.vector.tensor_scalar(
                            out=k1_sb[:, mt, :], in0=ps[:, half, 0:T2],
                            scalar1=bk1e[:, mt, ex:ex + 1], scalar2=0.0,
                            op0=OP.add, op1=OP.max)
                return run

            for mp in range(4):
                groups.append(k1_group(mp))

            def k2_group():
                ps = ps_mm.tile([128, 2, 512], f32, tag="mm")
                for kp in range(4):
                    nc.tensor.matmul(ps[0:CA, 0, 0:T2], w_k2[:, kp, :, :],
                                     k1_sb[:, 2 * kp:2 * kp + 2, :],
                                     start=(kp == 0), stop=(kp == 3), perf_mode=DR)
                nc.scalar.activation(out=k_s[0:CA, :], in_=ps[0:CA, 0, 0:T2],
                                     func=AF.Identity, scale=1.0 / (S1K * S2K),
                                     bias=b_k2c[:, 0:1])
            groups.append(k2_group)

            def k2n_group():
                nc.gpsimd.tensor_tensor(out=ksq[:], in0=k_s[0:CA, :],
                                        in1=k_s[0:CA, :], op=OP.mult)
                ps2 = ps_mm.tile([128, 2, 512], f32, tag="mm")
                nc.tensor.matmul(ps2[0:1, 0, 0:T2], ones_col[:, 0:1], ksq[:],
                                 start=True, stop=True)
                nc.vector.tensor_scalar(out=k2row[:], in0=ps2[0:1, 0, 0:T2],
                                        scalar1=-TEMP, scalar2=None, op0=OP.mult)
                nc.sync.dma_start(out=k_s[80:81, :], in_=k2row[:])
            groups.append(k2n_group)

            def q1_group(grp, sp):
                def run():
                    ps = ps_mm.tile([128, 2, 512], f32, tag="mm")
                    for sub in range(2):
                        base = (sp * 2 + sub) * 400
                        for dt in range(3):
                            nc.tensor.matmul(
                                ps[0:CM, sub, 0:400],
                                w_q1[:, dt, grp * 80:grp * 80 + 80],
                                q_sb[:, ex, dt + base:dt + base + 400],
                                start=(dt == 0), stop=(dt == 2))
                    nc.vector.tensor_scalar(
                        out=q1_sb[:, grp, sp * 800:sp * 800 + 800]
                        .rearrange("p (s t) -> p s t", s=2),
                        in0=ps[0:CM, :, 0:400],
                        scalar1=bq1e[0:CM, grp, ex:ex + 1], scalar2=0.0,
                        op0=OP.add, op1=OP.max)
                return run

            for grp in range(2):
                groups.append(q1_group(grp, 0))

            def q2_group(sp):
                def run():
                    ps = ps_mm.tile([128, 2, 512], f32, tag="mm")
                    for sub in range(2):
                        base = (sp * 2 + sub) * 400
                        nc.tensor.matmul(ps[0:CA, sub, 0:400], w_q2[:, :, :],
                                         q1_sb[:, 0:2, base:base + 400],
                                         start=True, stop=True, perf_mode=DR)
                    nc.scalar.activation(
                        out=q2_sb[:, sp * 800:sp * 800 + 800]
                        .rearrange("p (s t) -> p s t", s=2),
                        in_=ps[0:CA, :, 0:400], func=AF.Relu,
                        bias=b_q2c[:, 0:1])
                return run

            groups.append(q2_group(0))

            def q3_group(sp):
                def run():
                    ps = ps_mm.tile([128, 2, 512], f32, tag="mm")
                    for sub in range(2):
                        base = (sp * 2 + sub) * 400
                        nc.tensor.matmul(ps[0:CA, sub, 0:400], w_q3[:],
                                         q2_sb[:, base:base + 400],
                                         start=True, stop=True)
                    nc.scalar.activation(
                        out=q_s[0:CA, sp * 800:sp * 800 + 800]
                        .rearrange("p (s t) -> p s t", s=2),
                        in_=ps[0:CA, :, 0:400], func=AF.Identity,
                        scale=2.0 * TEMP / (S1Q * S2Q * S3Q),
                        bias=b_q3c[:, 0:1])
                return run

            groups.append(q3_group(0))
            tail = [q1_group(0, 1), q1_group(1, 1), q2_group(1), q3_group(1)]
            return groups, tail, (k_s, q_s)

        # ---------- attention for one example, pipelined + interleaved ----------
        CHUNKS = [(r0, 2, 128) for r0 in range(0, 1536, 256)] + [(1536, 1, 64)]

        def attention(ex, k_s, q_s, pending):
            st = {}

            def stage0(i):
                r0, cn, prow = CHUNKS[i]
                nrows = cn * prow
                t = {}
                t["pr"] = io.tile([128, 2, T2], bf, tag="pr", name="pr")
                t["e1"] = io.tile([128, 2, T2], f32, tag="e1", name="e1")
                t["g"] = io.tile([128, 2, T2], bf, tag="g", name="g")
                t["n"] = io.tile([128, 2, T2], bf, tag="n", name="n")
                t["lp"] = io.tile([128, 2, T2], f32, tag="lp", name="lp")
                t["at"] = io.tile([128, 2, T2], bf, tag="at", name="at")
                t["s1"] = sm.tile([128, 2], f32, tag="s1", name="s1")
                t["s2"] = sm.tile([128, 2], f32, tag="s2", name="s2")
                t["r1"] = sm.tile([128, 2], f32, tag="r1", name="r1")
                t["r1e"] = sm.tile([128, 2], f32, tag="r1e", name="r1e")
                t["r2"] = sm.tile([128, 2], f32, tag="r2", name="r2")
                st[i] = t
                nc.sync.dma_start(
                    out=t["pr"][0:prow, 0:cn, :],
                    in_=d_prior.ap()[ex, r0:r0 + nrows, :]
                    .rearrange("(p c) t -> p c t", c=cn))
                for c in range(cn):
                    ps = ps_at.tile([128, 512], f32, tag="att")
                    nc.tensor.matmul(ps[0:prow, 0:T2],
                                     q_s[:, r0 + c:r0 + nrows:cn],
                                     k_s[:], start=True, stop=True)
                    nc.scalar.activation(out=t["e1"][0:prow, c, :],
                                         in_=ps[0:prow, 0:T2], func=AF.Exp,
                                         accum_out=t["s1"][0:prow, c:c + 1])

            def stage1(i):
                r0, cn, prow = CHUNKS[i]
                t = st[i]
                nc.gpsimd.tensor_tensor(out=t["g"][0:prow, 0:cn, :],
                                        in0=t["e1"][0:prow, 0:cn, :],
                                        in1=t["pr"][0:prow, 0:cn, :], op=OP.mult)
                nc.vector.reciprocal(out=t["r1"][0:prow, 0:cn],
                                     in_=t["s1"][0:prow, 0:cn])
                nc.vector.tensor_scalar(out=t["r1e"][0:prow, 0:cn],
                                        in0=t["r1"][0:prow, 0:cn],
                                        scalar1=1e-8, scalar2=None, op0=OP.mult)

            def stage2(i):
                r0, cn, prow = CHUNKS[i]
                nrows = cn * prow
                t = st.pop(i)
                for c in range(cn):
                    nc.scalar.activation(out=t["lp"][0:prow, c, :],
                                         in_=t["g"][0:prow, c, :], func=AF.Ln,
                                         scale=t["r1"][0:prow, c:c + 1],
                                         bias=t["r1e"][0:prow, c:c + 1])
                    nc.vector.scalar_tensor_tensor(
                        out=t["n"][0:prow, c, :], in0=t["g"][0:prow, c, :],
                        scalar=1.0, in1=maskm[0:prow, ex, :],
                        op0=OP.mult, op1=OP.mult,
                        accum_out=t["s2"][0:prow, c:c + 1])
                nc.sync.dma_start(
                    out=d_lp.ap()[ex, r0:r0 + nrows, :]
                    .rearrange("(p c) t -> p c t", c=cn),
                    in_=t["lp"][0:prow, 0:cn, :])
                nc.vector.reciprocal(out=t["r2"][0:prow, 0:cn],
                                     in_=t["s2"][0:prow, 0:cn])
                for c in range(cn):
                    nc.vector.tensor_scalar(out=t["at"][0:prow, c, :],
                                            in0=t["n"][0:prow, c, :],
                                            scalar1=t["r2"][0:prow, c:c + 1],
                                            scalar2=None, op0=OP.mult)
                nc.sync.dma_start(
                    out=d_attn.ap()[ex, r0:r0 + nrows, :]
                    .rearrange("(p c) t -> p c t", c=cn),
                    in_=t["at"][0:prow, 0:cn, :])

            nch = len(CHUNKS)
            for i in range(nch + 2):
                if i < nch:
                    stage0(i)
                if 1 <= i < nch + 1:
                    stage1(i - 1)
                if i >= 2:
                    stage2(i - 2)
                # interleave ~2 conv groups of the next example per chunk
                for _ in range(2):
                    if pending:
                        pending.pop(0)()

        # ---------- main schedule ----------
        head0, tail0, tiles0 = conv_groups(0)
        for g in head0:
            g()
        cur = tiles0
        pending = list(tail0)
        for ex in range(BL):
            if ex + 1 < BL:
                nxt_head, nxt_tail, nxt_tiles = conv_groups(ex + 1)
                pending += nxt_head + nxt_tail
            else:
                nxt_tiles = None
            attention(ex, cur[0], cur[1], pending)
            for g in pending:
                g()
            pending = []
            cur = nxt_tiles

    nc.compile()
    return nc


def get_nc():
    if "nc" not in _CACHE:
        _CACHE["nc"] = _build_nc()
    return _CACHE["nc"]


def prep_in_maps(inputs):
    q = np.asarray(inputs["queries"], np.float32)
    k = np.asarray(inputs["keys"], np.float32)
    mask = np.asarray(inputs["mask"])
    prior = np.asarray(inputs["attn_prior"], np.float32)
    spk = np.asarray(inputs["speaker_embed"], np.float32)

    def f32(x):
        return np.ascontiguousarray(np.asarray(x, np.float32))

    def bf(x):
        return np.ascontiguousarray(np.asarray(x, np.float32).astype(BF16))

    def fp8(x):
        return np.ascontiguousarray(np.asarray(x, np.float32).astype(FP8))

    Wk1, bk1 = f32(inputs["Wk1"]), f32(inputs["bk1"])
    Wk2, bk2 = f32(inputs["Wk2"]), f32(inputs["bk2"])
    Wq1, bq1 = f32(inputs["Wq1"]), f32(inputs["bq1"])
    Wq2, bq2 = f32(inputs["Wq2"]), f32(inputs["bq2"])
    Wq3, bq3 = f32(inputs["Wq3"]), f32(inputs["bq3"])
    Wks, bks = f32(inputs["Wks"]), f32(inputs["bks"])
    Wqs, bqs = f32(inputs["Wqs"]), f32(inputs["bqs"])

    # speaker-bias folding: conv1(x + s) = conv1(x) + W1sum @ s
    W1ksum = Wk1.sum(axis=2)              # [1024, 512]
    Mk = W1ksum @ Wks                      # [1024, 512] (spk -> bias)
    b0k = bk1 + W1ksum @ bks               # [1024]
    W1qsum = Wq1.sum(axis=2)               # [160, 80]
    Mq = W1qsum @ Wqs                      # [160, 512]
    b0q = bq1 + W1qsum @ bqs               # [160]

    wk1 = fp8((S1K * Wk1).reshape(8, 128, 2, 2, 128, 3)
              .transpose(4, 2, 5, 3, 0, 1).reshape(128, 6, 2, 8, 128))
    wk2 = fp8((S2K * Wk2[:, :, 0]).reshape(CA, 4, 2, 128).transpose(3, 1, 2, 0))
    wq1 = bf((S1Q * Wq1).transpose(1, 2, 0))
    wq2 = fp8((S2Q * Wq2[:, :, 0]).T.reshape(2, CM, CA).transpose(1, 0, 2))
    wq3 = fp8((S3Q * Wq3[:, :, 0]).T)
    mk_h = fp8(Mk.T.reshape(4, 128, 1024).transpose(1, 0, 2))
    mq_h = bf(Mq.T.reshape(4, 128, 160).transpose(1, 0, 2))
    b0k_h = f32(S1K * b0k.reshape(8, 128).T)
    b0q_h = f32(S1Q * b0q.reshape(2, CM).T)
    bk2c = f32(bk2[:, None])
    bq2c = f32(S1Q * S2Q * bq2[:, None])
    bq3c = f32(2.0 * TEMP * bq3[:, None])

    q_t = np.zeros((B, CM, T1 + 2), np.float32)
    q_t[:, :, 1:T1 + 1] = q
    k_t = np.zeros((B, CT, T2 + 2), np.float32)
    k_t[:, :, 1:T2 + 1] = k
    q_bf = q_t.astype(BF16)
    k_f8 = k_t.astype(FP8)
    spk_bf = spk.astype(BF16)
    prior_bf = prior.astype(BF16)
    pmask = (~mask[:, :, 0]).astype(np.float32).astype(BF16)

    weights = dict(wk1=wk1, wk2=wk2, wq1=wq1, wq2=wq2, wq3=wq3,
                   mk=mk_h, mq=mq_h, b0k=b0k_h, b0q=b0q_h,
                   bk2c=bk2c, bq2c=bq2c, bq3c=bq3c)
    in_maps = []
    for c in range(N_CORES):
        sl = slice(c * BL, (c + 1) * BL)
        m = {"queries": np.ascontiguousarray(q_bf[sl]),
             "keys": np.ascontiguousarray(
                 k_f8[sl].reshape(BL, 4, 128, T2 + 2).transpose(0, 2, 1, 3)),
             "spk": np.ascontiguousarray(
                 spk_bf[sl].reshape(BL, 4, 128).transpose(2, 1, 0)),
             "prior": np.ascontiguousarray(prior_bf[sl]),
             "pmask": np.ascontiguousarray(pmask[sl][None])}
        m.update(weights)
        in_maps.append(m)
    return in_maps


def run_on_hw(inputs, trace=False, trace_kwargs=None):
    _ensure_paths()
    from concourse.bass_utils import run_bass_kernel_spmd
    nc = get_nc()
    in_maps = prep_in_maps(inputs)
    res = run_bass_kernel_spmd(nc, in_maps, core_ids=list(range(N_CORES)),
                               trace=trace, **(trace_kwargs or {}))
    attn = np.empty((B, 1, T1, T2), np.float32)
    lp = np.empty((B, 1, T1, T2), np.float32)
    for c in range(N_CORES):
        attn[c * BL:(c + 1) * BL, 0] = res.results[c]["attn"].astype(np.float32)
        lp[c * BL:(c + 1) * BL, 0] = res.results[c]["lp"].astype(np.float32)
    return (attn, lp), res


def kernel(**inputs):
    (attn, lp), _ = run_on_hw(inputs, trace=False)
    return attn, lp
